# revision 19
# baseline (speedup 1.0000x reference)
"""Causal self-attention (B=4, S=2048, D=1024, single head, fp32) on 8 trn2
NeuronCores.

Sharding: core 2*b + c handles batch b with the parity-c half of the keys
(global key rows 2*i + c), over ALL queries — a flash-attention split over
the key dimension. Each core returns unnormalized numerators o = sum_k
exp(s/sqrt(D)) v plus per-row l = sum exp (no row-max: logits are ~N(0,1),
fp32 exp cannot overflow); the host combines the two key-halves exactly.

Algebraic restructure: scores = (x Wq)(x Wk)^T = x (Wq Wk^T) x^T. The host
precomputes W_qk^T = Wk Wq^T once, and each core applies it KEY-side only:
u = x_keys W_qk^T  ->  scores^T[k, q] = u_k . x_q. This removes the q AND k
projections entirely (the q projection was also 2x-replicated across the
parity pair). Scores are produced transposed [key, query] so the exp'd
attn^T tile feeds the attn @ v matmul directly as the stationary operand —
no per-block PE transposes — and l = sum_k exp comes from a 1-column
ones-matmul.

Everything runs in fp16 (full PE rate, XBAR DMA transpose for x^T, 2x DVE
throughput); accumulation stays fp32 in PSUM, o is evicted and shipped f32.

SPMD trick: one program serves both parities. The host pair-swaps the rows
of x for odd cores (rows [1,0,3,2,...]), so each core's keys sit at even
row positions and the on-chip stride-2 access pattern is parity-free. The
causal boundary mask (which depends on the parity) ships as a small
per-core 0/1 input; the host pair-swaps the outputs of odd cores back.
"""
import math
import numpy as np

import concourse.bacc as bacc
import concourse.mybir as mybir
from concourse import tile
from concourse.bass_utils import run_bass_kernel_spmd

B, S, D = 4, 2048, 1024
P = 128
DT = D // P          # 8 d-tiles (feature)
ST = S // P          # 16 s-tiles
KT = S // 2 // P     # 8 compacted key tiles per core
NQB = S // P         # 16 query blocks
NJP = NQB // 2       # 8 query-block pairs (256 queries each)
INV_SQRT_D = 1.0 / math.sqrt(D)

F32 = mybir.dt.float32
F16 = mybir.dt.float16

_CACHED_NC = None


def build_nc():
    nc = bacc.Bacc("TRN2", target_bir_lowering=False)
    x_p = nc.declare_dram_parameter("x", [S, D], F16, isOutput=False)
    wqkT_p = nc.declare_dram_parameter("wqkT", [D, D], F16, isOutput=False)
    wv_p = nc.declare_dram_parameter("wv", [D, D], F16, isOutput=False)
    mask_p = nc.declare_dram_parameter("mask", [P, 2 * P], F16, isOutput=False)
    o_p = nc.declare_dram_parameter("o", [S, D], F16, isOutput=True)
    l_p = nc.declare_dram_parameter("l", [P, NQB], F32, isOutput=True)

    with tile.TileContext(nc) as tc:
        with (
            tc.tile_pool(name="xT_pool", bufs=1) as xT_pool,
            tc.tile_pool(name="uT_pool", bufs=1) as uT_pool,
            tc.tile_pool(name="v_pool", bufs=1) as v_pool,
            tc.tile_pool(name="const_pool", bufs=1) as const_pool,
        ):
            xT = xT_pool.tile([P, DT, S], F16)          # [d_p, dt, s] 32KB/p
            uT = uT_pool.tile([P, DT, KT * P], F16)     # [d_p, dt, k] 16KB/p
            vv = v_pool.tile([P, KT, D], F16)           # [k_p, kt, e] 16KB/p
            wqk_all = const_pool.tile([P, DT, D], F16)  # [d2_p, dt2, d1] 16KB/p
            wv_all = const_pool.tile([P, DT, D], F16)   # [d_p, dt, e] 16KB/p
            mask01 = const_pool.tile([P, 2 * P], F16)   # [k, q] 0/1
            ones_sb = const_pool.tile([P, 1], F16)
            l_all = const_pool.tile([P, NQB], F32)
            nc.gpsimd.memset(ones_sb[:], 1.0)
            # whole weight matrices, [contract-partition, strip, out] layout;
            # wv strips first (v is computed first), then wqk, then the mask
            for eb in range(2):
                for d in range(DT):
                    nc.scalar.dma_start(
                        out=wv_all[:, d, eb * 512:(eb + 1) * 512],
                        in_=wv_p[d * P:(d + 1) * P, eb * 512:(eb + 1) * 512])
            nc.scalar.dma_start(
                out=wqk_all[:],
                in_=wqkT_p.rearrange("(dt p) e -> p dt e", p=P))
            nc.scalar.dma_start(out=mask01[:], in_=mask_p[:])

            # even-position (this core's keys) stride-2 view of xT
            xT_keys = xT.rearrange("p d (s two) -> p d two s", two=2)

            # ================= Phase A: x^T, v, u =================
            with (
                tc.tile_pool(name="stage_pool", bufs=3) as stage_pool,
                tc.tile_pool(name="psA_all", bufs=1, space="PSUM") as psAll,
            ):
                psb = [psAll.tile([P, 512], F32, tag=f"b{i}", name=f"psb{i}")
                       for i in range(8)]

                # A1: x -> x^T. The host pre-applies the 32x32 block-grid
                # permute, so x lands via fast contiguous DMA and the DVE
                # stream transpose (32x32 blocks) finishes each tile straight
                # into xT — zero PE cost. Interleave v (A3) at every key-pair
                # so the PE ramps immediately.
                for st in range(ST):
                    x_sb = stage_pool.tile([P, DT, P], F16, tag="xs",
                                           name=f"xs{st}")
                    nc.sync.dma_start(out=x_sb[:],
                                      in_=x_p[st * P:(st + 1) * P, :])
                    for dti in range(DT):
                        nc.vector.transpose(xT[:, dti, st * P:(st + 1) * P],
                                            x_sb[:, dti, :])
                    if st % 2 == 1:
                        # A3: v[k, e] = sum_d x_key[k, d] * Wv[d, e]
                        kb = st // 2
                        for eb in range(2):
                            ps = psb[(kb * 2 + eb) % 8]
                            for d in range(DT):
                                nc.tensor.matmul(
                                    ps[:],
                                    xT_keys[:, d, 0, kb * P:(kb + 1) * P],
                                    wv_all[:, d, eb * 512:(eb + 1) * 512],
                                    start=(d == 0), stop=(d == DT - 1))
                            nc.vector.tensor_copy(
                                vv[:, kb, eb * 512:(eb + 1) * 512], ps[:])

                # A2: uT[d1, k] = sum_d2 W_qkT[d2, d1] * x_key[k, d2]
                for et in range(DT):
                    for ch in range(2):
                        ps = psb[(et * 2 + ch) % 8]
                        for d in range(DT):
                            nc.tensor.matmul(
                                ps[:],
                                wqk_all[:, d, et * P:(et + 1) * P],
                                xT_keys[:, d, 0, ch * 512:(ch + 1) * 512],
                                start=(d == 0), stop=(d == DT - 1))
                        nc.scalar.copy(uT[:, et, ch * 512:(ch + 1) * 512],
                                       ps[:])

            # ================= Phase B: causal attention =================
            with (
                tc.tile_pool(name="psS_pool", bufs=3, space="PSUM") as psS_pool,
                tc.tile_pool(name="psO_pool", bufs=1, space="PSUM") as psO_pool,
                tc.tile_pool(name="psL_pool", bufs=1, space="PSUM") as psL_pool,
                tc.tile_pool(name="at_pool", bufs=9) as at_pool,
                tc.tile_pool(name="ob_pool", bufs=2) as ob_pool,
            ):
                for jp in range(NJP):
                    q0 = jp * 2 * P
                    o_ps = [[psO_pool.tile([P, 512], F32, tag=f"o{h}{eb}",
                                           name=f"o{jp}_{h}{eb}")
                             for eb in range(2)] for h in range(2)]
                    l_ps = psL_pool.tile([P, 2], F32, tag="l", name=f"l{jp}")
                    at_tiles = []
                    for kb in range(jp + 1):
                        # scores^T[k, q] for 128 keys x 256 queries
                        sc_t = psS_pool.tile([P, 2 * P], F32, tag="sc",
                                             name=f"sc{jp}_{kb}")
                        for d in range(DT):
                            nc.tensor.matmul(
                                sc_t[:],
                                uT[:, d, kb * P:(kb + 1) * P],
                                xT[:, d, q0:q0 + 2 * P],
                                start=(d == 0), stop=(d == DT - 1))
                        at_t = at_pool.tile([P, 2 * P], F16, tag="at",
                                            name=f"at{jp}_{kb}")
                        nc.scalar.activation(
                            at_t[:], sc_t[:],
                            mybir.ActivationFunctionType.Exp,
                            scale=INV_SQRT_D)
                        if kb == jp:  # causal boundary: multiplicative mask
                            nc.gpsimd.tensor_mul(at_t[:], at_t[:], mask01[:])
                        at_tiles.append(at_t)
                        for h in range(2):
                            lhs = at_t[:, h * P:(h + 1) * P]
                            for eb in range(2):
                                nc.tensor.matmul(
                                    o_ps[h][eb][:],
                                    lhs,
                                    vv[:, kb, eb * 512:(eb + 1) * 512],
                                    start=(kb == 0), stop=(kb == jp),
                                    skip_group_check=True)
                    # l = sum_k attn: both column-groups share one PSUM bank,
                    # and a group's start=True clears has_written bits for the
                    # WHOLE bank — so the two groups must run sequentially,
                    # not interleaved.
                    for h in range(2):
                        for kb in range(jp + 1):
                            nc.tensor.matmul(
                                l_ps[:, h:h + 1],
                                at_tiles[kb][:, h * P:(h + 1) * P],
                                ones_sb[:],
                                start=(kb == 0), stop=(kb == jp),
                                skip_group_check=True)
                    for h in range(2):
                        for eb in range(2):
                            o_sb = ob_pool.tile([P, 512], F16, tag=f"ob{h}{eb}",
                                                name=f"ob{jp}_{h}{eb}")
                            if eb == 0:
                                nc.vector.tensor_copy(o_sb[:], o_ps[h][eb][:])
                            else:
                                nc.scalar.copy(o_sb[:], o_ps[h][eb][:])
                            nc.sync.dma_start(
                                out=o_p[q0 + h * P:q0 + (h + 1) * P,
                                        eb * 512:(eb + 1) * 512],
                                in_=o_sb[:])
                    nc.scalar.copy(l_all[:, 2 * jp:2 * jp + 2], l_ps[:])
                nc.sync.dma_start(out=l_p[:], in_=l_all[:])
    nc.finalize()
    return nc


def _boundary_mask(c):
    """mask01[i, q]: 1 if compacted key i (global row 2*i + c within the
    256-row-aligned block) is causally valid for local query position q of
    the same block, else 0.

    For parity-1 cores x arrives pair-swapped, so local position q holds
    global row q^1. Valid iff 2*i + c <= (q ^ c).
    """
    i = np.arange(P)[:, None]
    q = np.arange(2 * P)[None, :]
    return (2 * i + c <= (q ^ c)).astype(np.float16)


_PAIRSWAP = np.arange(S).reshape(-1, 2)[:, ::-1].reshape(-1)


def _make_in_maps(x, Wq, Wk, Wv):
    x = np.asarray(x, dtype=np.float32)
    Wq = np.asarray(Wq, dtype=np.float32)
    Wk = np.asarray(Wk, dtype=np.float32)
    Wv = np.asarray(Wv, dtype=np.float32)
    # scores = x (Wq Wk^T) x^T; key-side application needs W_qk^T = Wk Wq^T
    wqkT = np.ascontiguousarray((Wk @ Wq.T).astype(np.float16))
    wv16 = np.ascontiguousarray(Wv.astype(np.float16))
    masks = [_boundary_mask(0), _boundary_mask(1)]
    in_maps = []
    for core in range(8):
        b, c = core // 2, core % 2
        xb = x[b] if c == 0 else x[b][_PAIRSWAP]
        # pre-apply the 32x32 block-grid permute: the device DMAs rows
        # contiguously and the DVE stream transpose (32x32 blocks) then
        # yields the true transpose.
        xp = (xb.astype(np.float16)
              .reshape(ST, 4, 32, DT, 4, 32)
              .transpose(0, 4, 2, 3, 1, 5)
              .reshape(S, D))
        in_maps.append({
            "x": np.ascontiguousarray(xp),
            "wqkT": wqkT, "wv": wv16,
            "mask": masks[c],
        })
    return in_maps


def _combine(res):
    out = np.empty((B, S, D), dtype=np.float32)
    for b in range(B):
        r0, r1 = res.results[2 * b], res.results[2 * b + 1]

        def stat(r):
            return np.ascontiguousarray(r["l"].T).reshape(S, 1)
        o0 = r0["o"].astype(np.float64)
        l0 = stat(r0).astype(np.float64)
        o1 = r1["o"][_PAIRSWAP].astype(np.float64)
        l1 = stat(r1)[_PAIRSWAP].astype(np.float64)
        out[b] = ((o0 + o1) / (l0 + l1)).astype(np.float32)
    return out


def kernel(x, Wq, Wk, Wv):
    global _CACHED_NC
    if _CACHED_NC is None:
        _CACHED_NC = build_nc()
    in_maps = _make_in_maps(x, Wq, Wk, Wv)
    res = run_bass_kernel_spmd(_CACHED_NC, in_maps, list(range(8)))
    return _combine(res)


# revision 20
# speedup vs baseline: 1.0076x; 1.0076x over previous
"""Causal self-attention (B=4, S=2048, D=1024, single head, fp32) on 8 trn2
NeuronCores.

Sharding: core 2*b + c handles batch b with the parity-c half of the keys
(global key rows 2*i + c), over ALL queries — a flash-attention split over
the key dimension. Each core returns unnormalized numerators o = sum_k
exp(s/sqrt(D)) v plus per-row l = sum exp (no row-max: logits are ~N(0,1),
fp32 exp cannot overflow); the host combines the two key-halves exactly.

Algebraic restructure: scores = (x Wq)(x Wk)^T = x (Wq Wk^T) x^T. The host
precomputes W_qk^T = Wk Wq^T once, and each core applies it KEY-side only:
u = x_keys W_qk^T  ->  scores^T[k, q] = u_k . x_q. This removes the q AND k
projections entirely (the q projection was also 2x-replicated across the
parity pair). Scores are produced transposed [key, query] so the exp'd
attn^T tile feeds the attn @ v matmul directly as the stationary operand —
no per-block PE transposes — and l = sum_k exp comes from a 1-column
ones-matmul.

Everything runs in fp16 (full PE rate, XBAR DMA transpose for x^T, 2x DVE
throughput); accumulation stays fp32 in PSUM, o is evicted and shipped f32.

SPMD trick: one program serves both parities. The host pair-swaps the rows
of x for odd cores (rows [1,0,3,2,...]), so each core's keys sit at even
row positions and the on-chip stride-2 access pattern is parity-free. The
causal boundary mask (which depends on the parity) ships as a small
per-core 0/1 input; the host pair-swaps the outputs of odd cores back.
"""
import math
import numpy as np

import concourse.bacc as bacc
import concourse.mybir as mybir
from concourse import tile
from concourse.bass_utils import run_bass_kernel_spmd

B, S, D = 4, 2048, 1024
P = 128
DT = D // P          # 8 d-tiles (feature)
ST = S // P          # 16 s-tiles
KT = S // 2 // P     # 8 compacted key tiles per core
NQB = S // P         # 16 query blocks
NJP = NQB // 2       # 8 query-block pairs (256 queries each)
INV_SQRT_D = 1.0 / math.sqrt(D)

F32 = mybir.dt.float32
F16 = mybir.dt.float16

_CACHED_NC = None


def build_nc():
    nc = bacc.Bacc("TRN2", target_bir_lowering=False)
    x_p = nc.declare_dram_parameter("x", [S, D], F16, isOutput=False)
    wqkT_p = nc.declare_dram_parameter("wqkT", [D, D], F16, isOutput=False)
    wv_p = nc.declare_dram_parameter("wv", [D, D], F16, isOutput=False)
    mask_p = nc.declare_dram_parameter("mask", [P, 2 * P], F16, isOutput=False)
    o_p = nc.declare_dram_parameter("o", [S, D], F16, isOutput=True)
    l_p = nc.declare_dram_parameter("l", [P, NQB], F32, isOutput=True)

    with tile.TileContext(nc) as tc:
        with (
            tc.tile_pool(name="xT_pool", bufs=1) as xT_pool,
            tc.tile_pool(name="uT_pool", bufs=1) as uT_pool,
            tc.tile_pool(name="v_pool", bufs=1) as v_pool,
            tc.tile_pool(name="const_pool", bufs=1) as const_pool,
        ):
            xT = xT_pool.tile([P, DT, S], F16)          # [d_p, dt, s] 32KB/p
            uT = uT_pool.tile([P, DT, KT * P], F16)     # [d_p, dt, k] 16KB/p
            vv = v_pool.tile([P, KT, D], F16)           # [k_p, kt, e] 16KB/p
            wqk_all = const_pool.tile([P, DT, D], F16)  # [d2_p, dt2, d1] 16KB/p
            wv_all = const_pool.tile([P, DT, D], F16)   # [d_p, dt, e] 16KB/p
            mask01 = const_pool.tile([P, 2 * P], F16)   # [k, q] 0/1
            ones_sb = const_pool.tile([P, 1], F16)
            l_all = const_pool.tile([P, NQB], F32)
            nc.gpsimd.memset(ones_sb[:], 1.0)
            # whole weight matrices, [contract-partition, strip, out] layout;
            # wv strips first (v is computed first), then wqk, then the mask
            for eb in range(2):
                for d in range(DT):
                    nc.scalar.dma_start(
                        out=wv_all[:, d, eb * 512:(eb + 1) * 512],
                        in_=wv_p[d * P:(d + 1) * P, eb * 512:(eb + 1) * 512])
            nc.scalar.dma_start(
                out=wqk_all[:],
                in_=wqkT_p.rearrange("(dt p) e -> p dt e", p=P))
            nc.scalar.dma_start(out=mask01[:], in_=mask_p[:])

            # even-position (this core's keys) stride-2 view of xT
            xT_keys = xT.rearrange("p d (s two) -> p d two s", two=2)

            # ================= Phase A: x^T, v, u =================
            with (
                tc.tile_pool(name="stage_pool", bufs=3) as stage_pool,
                tc.tile_pool(name="psA_all", bufs=1, space="PSUM") as psAll,
            ):
                psb = [psAll.tile([P, 512], F32, tag=f"b{i}", name=f"psb{i}")
                       for i in range(8)]

                # A1: x -> x^T. The host pre-applies the 32x32 block-grid
                # permute, so x lands via fast contiguous DMA and the DVE
                # stream transpose (32x32 blocks) finishes each tile straight
                # into xT — zero PE cost. Interleave v (A3) at every key-pair
                # so the PE ramps immediately.
                for st in range(ST):
                    x_sb = stage_pool.tile([P, DT, P], F16, tag="xs",
                                           name=f"xs{st}")
                    nc.sync.dma_start(out=x_sb[:],
                                      in_=x_p[st * P:(st + 1) * P, :])
                    for dti in range(DT):
                        nc.vector.transpose(xT[:, dti, st * P:(st + 1) * P],
                                            x_sb[:, dti, :])
                    if st % 2 == 1:
                        # A3: v[k, e] = sum_d x_key[k, d] * Wv[d, e]
                        kb = st // 2
                        for eb in range(2):
                            ps = psb[(kb * 2 + eb) % 8]
                            for d in range(DT):
                                nc.tensor.matmul(
                                    ps[:],
                                    xT_keys[:, d, 0, kb * P:(kb + 1) * P],
                                    wv_all[:, d, eb * 512:(eb + 1) * 512],
                                    start=(d == 0), stop=(d == DT - 1))
                            nc.vector.tensor_copy(
                                vv[:, kb, eb * 512:(eb + 1) * 512], ps[:])

                # A2: uT[d1, k] = sum_d2 W_qkT[d2, d1] * x_key[k, d2]
                for et in range(DT):
                    for ch in range(2):
                        ps = psb[(et * 2 + ch) % 8]
                        for d in range(DT):
                            nc.tensor.matmul(
                                ps[:],
                                wqk_all[:, d, et * P:(et + 1) * P],
                                xT_keys[:, d, 0, ch * 512:(ch + 1) * 512],
                                start=(d == 0), stop=(d == DT - 1))
                        nc.scalar.copy(uT[:, et, ch * 512:(ch + 1) * 512],
                                       ps[:])

            # ================= Phase B: causal attention =================
            with (
                tc.tile_pool(name="psS_pool", bufs=3, space="PSUM") as psS_pool,
                tc.tile_pool(name="psO_pool", bufs=1, space="PSUM") as psO_pool,
                tc.tile_pool(name="psL_pool", bufs=1, space="PSUM") as psL_pool,
                tc.tile_pool(name="at_pool", bufs=9) as at_pool,
                tc.tile_pool(name="ob_pool", bufs=2) as ob_pool,
            ):
                for jp in range(NJP):
                    q0 = jp * 2 * P
                    o_ps = [[psO_pool.tile([P, 512], F32, tag=f"o{h}{eb}",
                                           name=f"o{jp}_{h}{eb}")
                             for eb in range(2)] for h in range(2)]
                    l_ps = psL_pool.tile([P, 2], F32, tag="l", name=f"l{jp}")
                    at_tiles = []
                    for kb in range(jp + 1):
                        # scores^T[k, q] for 128 keys x 256 queries
                        sc_t = psS_pool.tile([P, 2 * P], F32, tag="sc",
                                             name=f"sc{jp}_{kb}")
                        for d in range(DT):
                            nc.tensor.matmul(
                                sc_t[:],
                                uT[:, d, kb * P:(kb + 1) * P],
                                xT[:, d, q0:q0 + 2 * P],
                                start=(d == 0), stop=(d == DT - 1))
                        at_t = at_pool.tile([P, 2 * P], F16, tag="at",
                                            name=f"at{jp}_{kb}")
                        nc.scalar.activation(
                            at_t[:], sc_t[:],
                            mybir.ActivationFunctionType.Exp,
                            scale=INV_SQRT_D)
                        if kb == jp:  # causal boundary: multiplicative mask
                            nc.gpsimd.tensor_mul(at_t[:], at_t[:], mask01[:])
                        at_tiles.append(at_t)
                        for h in range(2):
                            lhs = at_t[:, h * P:(h + 1) * P]
                            for eb in range(2):
                                nc.tensor.matmul(
                                    o_ps[h][eb][:],
                                    lhs,
                                    vv[:, kb, eb * 512:(eb + 1) * 512],
                                    start=(kb == 0), stop=(kb == jp),
                                    skip_group_check=True)
                    # l = sum_k attn: both column-groups share one PSUM bank,
                    # and a group's start=True clears has_written bits for the
                    # WHOLE bank — so the two groups must run sequentially,
                    # not interleaved.
                    for h in range(2):
                        for kb in range(jp + 1):
                            nc.tensor.matmul(
                                l_ps[:, h:h + 1],
                                at_tiles[kb][:, h * P:(h + 1) * P],
                                ones_sb[:],
                                start=(kb == 0), stop=(kb == jp),
                                skip_group_check=True)
                    for h in range(2):
                        for eb in range(2):
                            o_sb = ob_pool.tile([P, 512], F16, tag=f"ob{h}{eb}",
                                                name=f"ob{jp}_{h}{eb}")
                            nc.vector.tensor_copy(o_sb[:], o_ps[h][eb][:])
                            nc.sync.dma_start(
                                out=o_p[q0 + h * P:q0 + (h + 1) * P,
                                        eb * 512:(eb + 1) * 512],
                                in_=o_sb[:])
                    nc.scalar.copy(l_all[:, 2 * jp:2 * jp + 2], l_ps[:])
                nc.sync.dma_start(out=l_p[:], in_=l_all[:])
    nc.finalize()
    return nc


def _boundary_mask(c):
    """mask01[i, q]: 1 if compacted key i (global row 2*i + c within the
    256-row-aligned block) is causally valid for local query position q of
    the same block, else 0.

    For parity-1 cores x arrives pair-swapped, so local position q holds
    global row q^1. Valid iff 2*i + c <= (q ^ c).
    """
    i = np.arange(P)[:, None]
    q = np.arange(2 * P)[None, :]
    return (2 * i + c <= (q ^ c)).astype(np.float16)


_PAIRSWAP = np.arange(S).reshape(-1, 2)[:, ::-1].reshape(-1)


def _make_in_maps(x, Wq, Wk, Wv):
    x = np.asarray(x, dtype=np.float32)
    Wq = np.asarray(Wq, dtype=np.float32)
    Wk = np.asarray(Wk, dtype=np.float32)
    Wv = np.asarray(Wv, dtype=np.float32)
    # scores = x (Wq Wk^T) x^T; key-side application needs W_qk^T = Wk Wq^T
    wqkT = np.ascontiguousarray((Wk @ Wq.T).astype(np.float16))
    wv16 = np.ascontiguousarray(Wv.astype(np.float16))
    masks = [_boundary_mask(0), _boundary_mask(1)]
    in_maps = []
    for core in range(8):
        b, c = core // 2, core % 2
        xb = x[b] if c == 0 else x[b][_PAIRSWAP]
        # pre-apply the 32x32 block-grid permute: the device DMAs rows
        # contiguously and the DVE stream transpose (32x32 blocks) then
        # yields the true transpose.
        xp = (xb.astype(np.float16)
              .reshape(ST, 4, 32, DT, 4, 32)
              .transpose(0, 4, 2, 3, 1, 5)
              .reshape(S, D))
        in_maps.append({
            "x": np.ascontiguousarray(xp),
            "wqkT": wqkT, "wv": wv16,
            "mask": masks[c],
        })
    return in_maps


def _combine(res):
    out = np.empty((B, S, D), dtype=np.float32)
    for b in range(B):
        r0, r1 = res.results[2 * b], res.results[2 * b + 1]

        def stat(r):
            return np.ascontiguousarray(r["l"].T).reshape(S, 1)
        o0 = r0["o"].astype(np.float64)
        l0 = stat(r0).astype(np.float64)
        o1 = r1["o"][_PAIRSWAP].astype(np.float64)
        l1 = stat(r1)[_PAIRSWAP].astype(np.float64)
        out[b] = ((o0 + o1) / (l0 + l1)).astype(np.float32)
    return out


def kernel(x, Wq, Wk, Wv):
    global _CACHED_NC
    if _CACHED_NC is None:
        _CACHED_NC = build_nc()
    in_maps = _make_in_maps(x, Wq, Wk, Wv)
    res = run_bass_kernel_spmd(_CACHED_NC, in_maps, list(range(8)))
    return _combine(res)


# revision 21
# speedup vs baseline: 1.0091x; 1.0015x over previous
"""Causal self-attention (B=4, S=2048, D=1024, single head, fp32) on 8 trn2
NeuronCores.

Sharding: core 2*b + c handles batch b with the parity-c half of the keys
(global key rows 2*i + c), over ALL queries — a flash-attention split over
the key dimension. Each core returns unnormalized numerators o = sum_k
exp(s/sqrt(D)) v plus per-row l = sum exp (no row-max: logits are ~N(0,1),
fp32 exp cannot overflow); the host combines the two key-halves exactly.

Algebraic restructure: scores = (x Wq)(x Wk)^T = x (Wq Wk^T) x^T. The host
precomputes W_qk^T = Wk Wq^T once, and each core applies it KEY-side only:
u = x_keys W_qk^T  ->  scores^T[k, q] = u_k . x_q. This removes the q AND k
projections entirely (the q projection was also 2x-replicated across the
parity pair). Scores are produced transposed [key, query] so the exp'd
attn^T tile feeds the attn @ v matmul directly as the stationary operand —
no per-block PE transposes — and l = sum_k exp comes from a 1-column
ones-matmul.

Everything runs in fp16 (full PE rate, XBAR DMA transpose for x^T, 2x DVE
throughput); accumulation stays fp32 in PSUM, o is evicted and shipped f32.

SPMD trick: one program serves both parities. The host pair-swaps the rows
of x for odd cores (rows [1,0,3,2,...]), so each core's keys sit at even
row positions and the on-chip stride-2 access pattern is parity-free. The
causal boundary mask (which depends on the parity) ships as a small
per-core 0/1 input; the host pair-swaps the outputs of odd cores back.
"""
import math
import numpy as np

import concourse.bacc as bacc
import concourse.mybir as mybir
from concourse import tile
from concourse.bass_utils import run_bass_kernel_spmd

B, S, D = 4, 2048, 1024
P = 128
DT = D // P          # 8 d-tiles (feature)
ST = S // P          # 16 s-tiles
KT = S // 2 // P     # 8 compacted key tiles per core
NQB = S // P         # 16 query blocks
NJP = NQB // 2       # 8 query-block pairs (256 queries each)
INV_SQRT_D = 1.0 / math.sqrt(D)

F32 = mybir.dt.float32
F16 = mybir.dt.float16

_CACHED_NC = None


def build_nc():
    nc = bacc.Bacc("TRN2", target_bir_lowering=False)
    x_p = nc.declare_dram_parameter("x", [S, D], F16, isOutput=False)
    wqkT_p = nc.declare_dram_parameter("wqkT", [D, D], F16, isOutput=False)
    wv_p = nc.declare_dram_parameter("wv", [D, D], F16, isOutput=False)
    mask_p = nc.declare_dram_parameter("mask", [P, 2 * P], F16, isOutput=False)
    o_p = nc.declare_dram_parameter("o", [S, D], F32, isOutput=True)
    l_p = nc.declare_dram_parameter("l", [P, NQB], F32, isOutput=True)

    with tile.TileContext(nc) as tc:
        with (
            tc.tile_pool(name="xT_pool", bufs=1) as xT_pool,
            tc.tile_pool(name="uT_pool", bufs=1) as uT_pool,
            tc.tile_pool(name="v_pool", bufs=1) as v_pool,
            tc.tile_pool(name="const_pool", bufs=1) as const_pool,
        ):
            xT = xT_pool.tile([P, DT, S], F16)          # [d_p, dt, s] 32KB/p
            uT = uT_pool.tile([P, DT, KT * P], F16)     # [d_p, dt, k] 16KB/p
            vv = v_pool.tile([P, KT, D], F16)           # [k_p, kt, e] 16KB/p
            wqk_all = const_pool.tile([P, DT, D], F16)  # [d2_p, dt2, d1] 16KB/p
            wv_all = const_pool.tile([P, DT, D], F16)   # [d_p, dt, e] 16KB/p
            mask01 = const_pool.tile([P, 2 * P], F16)   # [k, q] 0/1
            ones_sb = const_pool.tile([P, 1], F16)
            l_all = const_pool.tile([P, NQB], F32)
            nc.gpsimd.memset(ones_sb[:], 1.0)
            # whole weight matrices, [contract-partition, strip, out] layout;
            # wv strips first (v is computed first), then wqk, then the mask
            for eb in range(2):
                for d in range(DT):
                    nc.scalar.dma_start(
                        out=wv_all[:, d, eb * 512:(eb + 1) * 512],
                        in_=wv_p[d * P:(d + 1) * P, eb * 512:(eb + 1) * 512])
            nc.scalar.dma_start(
                out=wqk_all[:],
                in_=wqkT_p.rearrange("(dt p) e -> p dt e", p=P))
            nc.scalar.dma_start(out=mask01[:], in_=mask_p[:])

            # even-position (this core's keys) stride-2 view of xT
            xT_keys = xT.rearrange("p d (s two) -> p d two s", two=2)

            # ================= Phase A: x^T, v, u =================
            with (
                tc.tile_pool(name="stage_pool", bufs=3) as stage_pool,
                tc.tile_pool(name="psA_all", bufs=1, space="PSUM") as psAll,
            ):
                psb = [psAll.tile([P, 512], F32, tag=f"b{i}", name=f"psb{i}")
                       for i in range(8)]

                # A1: x -> x^T. The host pre-applies the 32x32 block-grid
                # permute, so x lands via fast contiguous DMA and the DVE
                # stream transpose (32x32 blocks) finishes each tile straight
                # into xT — zero PE cost. Interleave v (A3) at every key-pair
                # so the PE ramps immediately.
                for st in range(ST):
                    x_sb = stage_pool.tile([P, DT, P], F16, tag="xs",
                                           name=f"xs{st}")
                    nc.sync.dma_start(out=x_sb[:],
                                      in_=x_p[st * P:(st + 1) * P, :])
                    for dti in range(DT):
                        nc.vector.transpose(xT[:, dti, st * P:(st + 1) * P],
                                            x_sb[:, dti, :])
                    if st % 2 == 1:
                        # A3: v[k, e] = sum_d x_key[k, d] * Wv[d, e]
                        kb = st // 2
                        for eb in range(2):
                            ps = psb[(kb * 2 + eb) % 8]
                            for d in range(DT):
                                nc.tensor.matmul(
                                    ps[:],
                                    xT_keys[:, d, 0, kb * P:(kb + 1) * P],
                                    wv_all[:, d, eb * 512:(eb + 1) * 512],
                                    start=(d == 0), stop=(d == DT - 1))
                            nc.vector.tensor_copy(
                                vv[:, kb, eb * 512:(eb + 1) * 512], ps[:])

                # A2: uT[d1, k] = sum_d2 W_qkT[d2, d1] * x_key[k, d2]
                for et in range(DT):
                    for ch in range(2):
                        ps = psb[(et * 2 + ch) % 8]
                        for d in range(DT):
                            nc.tensor.matmul(
                                ps[:],
                                wqk_all[:, d, et * P:(et + 1) * P],
                                xT_keys[:, d, 0, ch * 512:(ch + 1) * 512],
                                start=(d == 0), stop=(d == DT - 1))
                        nc.scalar.copy(uT[:, et, ch * 512:(ch + 1) * 512],
                                       ps[:])

            # ================= Phase B: causal attention =================
            with (
                tc.tile_pool(name="psS_pool", bufs=3, space="PSUM") as psS_pool,
                tc.tile_pool(name="psO_pool", bufs=1, space="PSUM") as psO_pool,
                tc.tile_pool(name="psL_pool", bufs=1, space="PSUM") as psL_pool,
                tc.tile_pool(name="at_pool", bufs=10) as at_pool,
                tc.tile_pool(name="ob_pool", bufs=2) as ob_pool,
            ):
                for jp in range(NJP):
                    q0 = jp * 2 * P
                    o_ps = [[psO_pool.tile([P, 512], F32, tag=f"o{h}{eb}",
                                           name=f"o{jp}_{h}{eb}")
                             for eb in range(2)] for h in range(2)]
                    l_ps = psL_pool.tile([P, 2], F32, tag="l", name=f"l{jp}")
                    at_tiles = {}
                    kbs = ([jp] + list(range(jp))) if jp > 0 else [0]
                    for i, kb in enumerate(kbs):
                        # scores^T[k, q] for 128 keys x 256 queries
                        sc_t = psS_pool.tile([P, 2 * P], F32, tag="sc",
                                             name=f"sc{jp}_{kb}")
                        for d in range(DT):
                            nc.tensor.matmul(
                                sc_t[:],
                                uT[:, d, kb * P:(kb + 1) * P],
                                xT[:, d, q0:q0 + 2 * P],
                                start=(d == 0), stop=(d == DT - 1))
                        at_t = at_pool.tile([P, 2 * P], F16, tag="at",
                                            name=f"at{jp}_{kb}")
                        nc.scalar.activation(
                            at_t[:], sc_t[:],
                            mybir.ActivationFunctionType.Exp,
                            scale=INV_SQRT_D)
                        if kb == jp:  # causal boundary: multiplicative mask
                            nc.gpsimd.tensor_mul(at_t[:], at_t[:], mask01[:])
                        at_tiles[kb] = at_t
                        for h in range(2):
                            lhs = at_t[:, h * P:(h + 1) * P]
                            for eb in range(2):
                                nc.tensor.matmul(
                                    o_ps[h][eb][:],
                                    lhs,
                                    vv[:, kb, eb * 512:(eb + 1) * 512],
                                    start=(i == 0), stop=(i == jp),
                                    skip_group_check=True)
                    # l = sum_k attn: both column-groups share one PSUM bank,
                    # and a group's start=True clears has_written bits for the
                    # WHOLE bank — so the two groups must run sequentially,
                    # not interleaved.
                    for h in range(2):
                        for i, kb in enumerate(kbs):
                            nc.tensor.matmul(
                                l_ps[:, h:h + 1],
                                at_tiles[kb][:, h * P:(h + 1) * P],
                                ones_sb[:],
                                start=(i == 0), stop=(i == jp),
                                skip_group_check=True)
                    for h in range(2):
                        for eb in range(2):
                            o_sb = ob_pool.tile([P, 512], F32, tag=f"ob{h}{eb}",
                                                name=f"ob{jp}_{h}{eb}")
                            nc.vector.tensor_copy(o_sb[:], o_ps[h][eb][:])
                            nc.sync.dma_start(
                                out=o_p[q0 + h * P:q0 + (h + 1) * P,
                                        eb * 512:(eb + 1) * 512],
                                in_=o_sb[:])
                    nc.scalar.copy(l_all[:, 2 * jp:2 * jp + 2], l_ps[:])
                nc.sync.dma_start(out=l_p[:], in_=l_all[:])
    nc.finalize()
    return nc


def _boundary_mask(c):
    """mask01[i, q]: 1 if compacted key i (global row 2*i + c within the
    256-row-aligned block) is causally valid for local query position q of
    the same block, else 0.

    For parity-1 cores x arrives pair-swapped, so local position q holds
    global row q^1. Valid iff 2*i + c <= (q ^ c).
    """
    i = np.arange(P)[:, None]
    q = np.arange(2 * P)[None, :]
    return (2 * i + c <= (q ^ c)).astype(np.float16)


_PAIRSWAP = np.arange(S).reshape(-1, 2)[:, ::-1].reshape(-1)


def _make_in_maps(x, Wq, Wk, Wv):
    x = np.asarray(x, dtype=np.float32)
    Wq = np.asarray(Wq, dtype=np.float32)
    Wk = np.asarray(Wk, dtype=np.float32)
    Wv = np.asarray(Wv, dtype=np.float32)
    # scores = x (Wq Wk^T) x^T; key-side application needs W_qk^T = Wk Wq^T
    wqkT = np.ascontiguousarray((Wk @ Wq.T).astype(np.float16))
    wv16 = np.ascontiguousarray(Wv.astype(np.float16))
    masks = [_boundary_mask(0), _boundary_mask(1)]
    in_maps = []
    for core in range(8):
        b, c = core // 2, core % 2
        xb = x[b] if c == 0 else x[b][_PAIRSWAP]
        # pre-apply the 32x32 block-grid permute: the device DMAs rows
        # contiguously and the DVE stream transpose (32x32 blocks) then
        # yields the true transpose.
        xp = (xb.astype(np.float16)
              .reshape(ST, 4, 32, DT, 4, 32)
              .transpose(0, 4, 2, 3, 1, 5)
              .reshape(S, D))
        in_maps.append({
            "x": np.ascontiguousarray(xp),
            "wqkT": wqkT, "wv": wv16,
            "mask": masks[c],
        })
    return in_maps


def _combine(res):
    out = np.empty((B, S, D), dtype=np.float32)
    for b in range(B):
        r0, r1 = res.results[2 * b], res.results[2 * b + 1]

        def stat(r):
            return np.ascontiguousarray(r["l"].T).reshape(S, 1)
        o0 = r0["o"].astype(np.float64)
        l0 = stat(r0).astype(np.float64)
        o1 = r1["o"][_PAIRSWAP].astype(np.float64)
        l1 = stat(r1)[_PAIRSWAP].astype(np.float64)
        out[b] = ((o0 + o1) / (l0 + l1)).astype(np.float32)
    return out


def kernel(x, Wq, Wk, Wv):
    global _CACHED_NC
    if _CACHED_NC is None:
        _CACHED_NC = build_nc()
    in_maps = _make_in_maps(x, Wq, Wk, Wv)
    res = run_bass_kernel_spmd(_CACHED_NC, in_maps, list(range(8)))
    return _combine(res)


# revision 22
# speedup vs baseline: 1.0100x; 1.0009x over previous
"""Causal self-attention (B=4, S=2048, D=1024, single head, fp32) on 8 trn2
NeuronCores.

Sharding: core 2*b + c handles batch b with the parity-c half of the keys
(global key rows 2*i + c), over ALL queries — a flash-attention split over
the key dimension. Each core returns unnormalized numerators o = sum_k
exp(s/sqrt(D)) v plus per-row l = sum exp (no row-max: logits are ~N(0,1),
fp32 exp cannot overflow); the host combines the two key-halves exactly.

Algebraic restructure: scores = (x Wq)(x Wk)^T = x (Wq Wk^T) x^T. The host
precomputes W_qk^T = Wk Wq^T once, and each core applies it KEY-side only:
u = x_keys W_qk^T  ->  scores^T[k, q] = u_k . x_q. This removes the q AND k
projections entirely (the q projection was also 2x-replicated across the
parity pair). Scores are produced transposed [key, query] so the exp'd
attn^T tile feeds the attn @ v matmul directly as the stationary operand —
no per-block PE transposes — and l = sum_k exp comes from a 1-column
ones-matmul.

Everything runs in fp16 (full PE rate, 2x DVE throughput); accumulation
stays fp32 in PSUM, o is evicted and shipped f32. x^T is built with zero PE
cost: the host pre-applies a 32x32 block-grid permute so x lands via fast
contiguous DMA and the DVE stream transpose (32x32 blocks) completes the
transpose. (The XBAR DMA-transpose instruction is NOT used: it fans out
over all 16 chip DMA engines and silently drops tiles when all 8 cores
issue it concurrently.)

SPMD trick: one program serves both parities. The host pair-swaps the rows
of x for odd cores (rows [1,0,3,2,...]), so each core's keys sit at even
row positions and the on-chip stride-2 access pattern is parity-free. The
causal boundary mask (which depends on the parity) ships as a small
per-core 0/1 input; the host pair-swaps the outputs of odd cores back.
"""
import math
import numpy as np

import concourse.bacc as bacc
import concourse.mybir as mybir
from concourse import tile
from concourse.bass_utils import run_bass_kernel_spmd

B, S, D = 4, 2048, 1024
P = 128
DT = D // P          # 8 d-tiles (feature)
ST = S // P          # 16 s-tiles
KT = S // 2 // P     # 8 compacted key tiles per core
NQB = S // P         # 16 query blocks
NJP = NQB // 2       # 8 query-block pairs (256 queries each)
INV_SQRT_D = 1.0 / math.sqrt(D)

F32 = mybir.dt.float32
F16 = mybir.dt.float16

_CACHED_NC = None


def build_nc():
    nc = bacc.Bacc("TRN2", target_bir_lowering=False)
    x_p = nc.declare_dram_parameter("x", [S, D], F16, isOutput=False)
    wqkT_p = nc.declare_dram_parameter("wqkT", [D, D], F16, isOutput=False)
    wv_p = nc.declare_dram_parameter("wv", [D, D], F16, isOutput=False)
    mask_p = nc.declare_dram_parameter("mask", [P, 2 * P], F16, isOutput=False)
    o_p = nc.declare_dram_parameter("o", [S, D], F32, isOutput=True)
    l_p = nc.declare_dram_parameter("l", [P, NQB], F32, isOutput=True)

    with tile.TileContext(nc) as tc:
        with (
            tc.tile_pool(name="xT_pool", bufs=1) as xT_pool,
            tc.tile_pool(name="uT_pool", bufs=1) as uT_pool,
            tc.tile_pool(name="v_pool", bufs=1) as v_pool,
            tc.tile_pool(name="const_pool", bufs=1) as const_pool,
        ):
            xT = xT_pool.tile([P, DT, S], F16)          # [d_p, dt, s] 32KB/p
            uT = uT_pool.tile([P, DT, KT * P], F16)     # [d_p, dt, k] 16KB/p
            vv = v_pool.tile([P, KT, D], F16)           # [k_p, kt, e] 16KB/p
            wqk_all = const_pool.tile([P, DT, D], F16)  # [d2_p, dt2, d1] 16KB/p
            wv_all = const_pool.tile([P, DT, D], F16)   # [d_p, dt, e] 16KB/p
            mask01 = const_pool.tile([P, 2 * P], F16)   # [k, q] 0/1
            ones_sb = const_pool.tile([P, 1], F16)
            l_all = const_pool.tile([P, NQB], F32)
            nc.gpsimd.memset(ones_sb[:], 1.0)
            # whole weight matrices, [contract-partition, strip, out] layout;
            # wv strips first (v is computed first), then wqk, then the mask
            for eb in range(2):
                for d in range(DT):
                    nc.scalar.dma_start(
                        out=wv_all[:, d, eb * 512:(eb + 1) * 512],
                        in_=wv_p[d * P:(d + 1) * P, eb * 512:(eb + 1) * 512])
            nc.scalar.dma_start(
                out=wqk_all[:],
                in_=wqkT_p.rearrange("(dt p) e -> p dt e", p=P))
            nc.scalar.dma_start(out=mask01[:], in_=mask_p[:])

            # even-position (this core's keys) stride-2 view of xT
            xT_keys = xT.rearrange("p d (s two) -> p d two s", two=2)

            # ================= Phase A: x^T, v, u =================
            with (
                tc.tile_pool(name="stage_pool", bufs=3) as stage_pool,
                tc.tile_pool(name="psA_all", bufs=1, space="PSUM") as psAll,
            ):
                psb = [psAll.tile([P, 512], F32, tag=f"b{i}", name=f"psb{i}")
                       for i in range(8)]

                # A1: x -> x^T. The host pre-applies the 32x32 block-grid
                # permute, so x lands via fast contiguous DMA and the DVE
                # stream transpose (32x32 blocks) finishes each tile straight
                # into xT — zero PE cost. Interleave v (A3) at every key-pair
                # so the PE ramps immediately.
                for st in range(ST):
                    x_sb = stage_pool.tile([P, DT, P], F16, tag="xs",
                                           name=f"xs{st}")
                    nc.sync.dma_start(out=x_sb[:],
                                      in_=x_p[st * P:(st + 1) * P, :])
                    for dti in range(DT):
                        nc.vector.transpose(xT[:, dti, st * P:(st + 1) * P],
                                            x_sb[:, dti, :])
                    if st % 2 == 1:
                        # A3: v[k, e] = sum_d x_key[k, d] * Wv[d, e]
                        kb = st // 2
                        for eb in range(2):
                            ps = psb[(kb * 2 + eb) % 8]
                            for d in range(DT):
                                nc.tensor.matmul(
                                    ps[:],
                                    xT_keys[:, d, 0, kb * P:(kb + 1) * P],
                                    wv_all[:, d, eb * 512:(eb + 1) * 512],
                                    start=(d == 0), stop=(d == DT - 1))
                            nc.vector.tensor_copy(
                                vv[:, kb, eb * 512:(eb + 1) * 512], ps[:])

                # A2: uT[d1, k] = sum_d2 W_qkT[d2, d1] * x_key[k, d2]
                for et in range(DT):
                    for ch in range(2):
                        ps = psb[(et * 2 + ch) % 8]
                        for d in range(DT):
                            nc.tensor.matmul(
                                ps[:],
                                wqk_all[:, d, et * P:(et + 1) * P],
                                xT_keys[:, d, 0, ch * 512:(ch + 1) * 512],
                                start=(d == 0), stop=(d == DT - 1))
                        nc.scalar.copy(uT[:, et, ch * 512:(ch + 1) * 512],
                                       ps[:])

            # ================= Phase B: causal attention =================
            with (
                tc.tile_pool(name="psS_pool", bufs=3, space="PSUM") as psS_pool,
                tc.tile_pool(name="psO_pool", bufs=1, space="PSUM") as psO_pool,
                tc.tile_pool(name="psL_pool", bufs=1, space="PSUM") as psL_pool,
                tc.tile_pool(name="at_pool", bufs=10) as at_pool,
                tc.tile_pool(name="ob_pool", bufs=2) as ob_pool,
            ):
                for jp in range(NJP):
                    q0 = jp * 2 * P
                    o_ps = [[psO_pool.tile([P, 512], F32, tag=f"o{h}{eb}",
                                           name=f"o{jp}_{h}{eb}")
                             for eb in range(2)] for h in range(2)]
                    l_ps = psL_pool.tile([P, 2], F32, tag="l", name=f"l{jp}")
                    at_tiles = {}
                    kbs = ([jp] + list(range(jp))) if jp > 0 else [0]
                    for i, kb in enumerate(kbs):
                        # scores^T[k, q] for 128 keys x 256 queries
                        sc_t = psS_pool.tile([P, 2 * P], F32, tag="sc",
                                             name=f"sc{jp}_{kb}")
                        for d in range(DT):
                            nc.tensor.matmul(
                                sc_t[:],
                                uT[:, d, kb * P:(kb + 1) * P],
                                xT[:, d, q0:q0 + 2 * P],
                                start=(d == 0), stop=(d == DT - 1))
                        at_t = at_pool.tile([P, 2 * P], F16, tag="at",
                                            name=f"at{jp}_{kb}")
                        nc.scalar.activation(
                            at_t[:], sc_t[:],
                            mybir.ActivationFunctionType.Exp,
                            scale=INV_SQRT_D)
                        if kb == jp:  # causal boundary: multiplicative mask
                            nc.gpsimd.tensor_mul(at_t[:], at_t[:], mask01[:])
                        at_tiles[kb] = at_t
                        for h in range(2):
                            lhs = at_t[:, h * P:(h + 1) * P]
                            for eb in range(2):
                                nc.tensor.matmul(
                                    o_ps[h][eb][:],
                                    lhs,
                                    vv[:, kb, eb * 512:(eb + 1) * 512],
                                    start=(i == 0), stop=(i == jp),
                                    skip_group_check=True)
                    # l = sum_k attn: both column-groups share one PSUM bank,
                    # and a group's start=True clears has_written bits for the
                    # WHOLE bank — so the two groups must run sequentially,
                    # not interleaved.
                    for h in range(2):
                        for i, kb in enumerate(kbs):
                            nc.tensor.matmul(
                                l_ps[:, h:h + 1],
                                at_tiles[kb][:, h * P:(h + 1) * P],
                                ones_sb[:],
                                start=(i == 0), stop=(i == jp),
                                skip_group_check=True)
                    for h in range(2):
                        for eb in range(2):
                            o_sb = ob_pool.tile([P, 512], F32, tag=f"ob{h}{eb}",
                                                name=f"ob{jp}_{h}{eb}")
                            nc.vector.tensor_copy(o_sb[:], o_ps[h][eb][:])
                            nc.sync.dma_start(
                                out=o_p[q0 + h * P:q0 + (h + 1) * P,
                                        eb * 512:(eb + 1) * 512],
                                in_=o_sb[:])
                    nc.scalar.copy(l_all[:, 2 * jp:2 * jp + 2], l_ps[:])
                nc.sync.dma_start(out=l_p[:], in_=l_all[:])
    nc.finalize()
    return nc


def _boundary_mask(c):
    """mask01[i, q]: 1 if compacted key i (global row 2*i + c within the
    256-row-aligned block) is causally valid for local query position q of
    the same block, else 0.

    For parity-1 cores x arrives pair-swapped, so local position q holds
    global row q^1. Valid iff 2*i + c <= (q ^ c).
    """
    i = np.arange(P)[:, None]
    q = np.arange(2 * P)[None, :]
    return (2 * i + c <= (q ^ c)).astype(np.float16)


_PAIRSWAP = np.arange(S).reshape(-1, 2)[:, ::-1].reshape(-1)


def _make_in_maps(x, Wq, Wk, Wv):
    x = np.asarray(x, dtype=np.float32)
    Wq = np.asarray(Wq, dtype=np.float32)
    Wk = np.asarray(Wk, dtype=np.float32)
    Wv = np.asarray(Wv, dtype=np.float32)
    # scores = x (Wq Wk^T) x^T; key-side application needs W_qk^T = Wk Wq^T
    wqkT = np.ascontiguousarray((Wk @ Wq.T).astype(np.float16))
    wv16 = np.ascontiguousarray(Wv.astype(np.float16))
    masks = [_boundary_mask(0), _boundary_mask(1)]
    in_maps = []
    for core in range(8):
        b, c = core // 2, core % 2
        xb = x[b] if c == 0 else x[b][_PAIRSWAP]
        # pre-apply the 32x32 block-grid permute: the device DMAs rows
        # contiguously and the DVE stream transpose (32x32 blocks) then
        # yields the true transpose.
        xp = (xb.astype(np.float16)
              .reshape(ST, 4, 32, DT, 4, 32)
              .transpose(0, 4, 2, 3, 1, 5)
              .reshape(S, D))
        in_maps.append({
            "x": np.ascontiguousarray(xp),
            "wqkT": wqkT, "wv": wv16,
            "mask": masks[c],
        })
    return in_maps


def _combine(res):
    out = np.empty((B, S, D), dtype=np.float32)
    for b in range(B):
        r0, r1 = res.results[2 * b], res.results[2 * b + 1]

        def stat(r):
            return np.ascontiguousarray(r["l"].T).reshape(S, 1)
        o0 = r0["o"].astype(np.float64)
        l0 = stat(r0).astype(np.float64)
        o1 = r1["o"][_PAIRSWAP].astype(np.float64)
        l1 = stat(r1)[_PAIRSWAP].astype(np.float64)
        out[b] = ((o0 + o1) / (l0 + l1)).astype(np.float32)
    return out


def kernel(x, Wq, Wk, Wv):
    global _CACHED_NC
    if _CACHED_NC is None:
        _CACHED_NC = build_nc()
    in_maps = _make_in_maps(x, Wq, Wk, Wv)
    res = run_bass_kernel_spmd(_CACHED_NC, in_maps, list(range(8)))
    return _combine(res)
